# revision 1
# baseline (speedup 1.0000x reference)
"""BandSplit (BSRNN) Trainium2 kernel.

Math per band k (31 bands over 257 freq bins, band widths 3/6/16/27):
  xg = x[b, :, band_bins, t] flattened to d = 2*bw features (torch order:
       bin-major, re/im minor)
  out[b, k, t, :] = LayerNorm_d(xg) @ W_k + b_k          (d -> C=128)

Algebraic refactor (per band, per t), with q = x * rstd:
  out = q @ Wg - mean_d(q) * (sum_d Wg) + bb
      = q @ (Wg - colmean_d(Wg)) + bb
  with host-precomputed  Wg = gamma*W,  bb = b + beta @ W.
So the device only needs rstd (from mu/meansq), q = x*rstd, and one
matmul per (pack, t-chunk) whose lhsT is [q ; ones] and whose rhs is
the blockdiag centered Wg with a bb row. All matmuls stream float32r
(bitcast views) so the PE runs at 1 cycle/row.

Sharding: batch-parallel, core b handles x[b] (B=8 = n_cores).
"""

import numpy as np

T = 3000
C = 128
F_BINS = 257
EPS = 1e-5
GROUPS = [(10, 3), (12, 6), (8, 16), (1, 27)]  # (n_bands, bins_per_band)

SPAN = 512   # stats/prep span (free dim of PSUM bank) == x-slab width
CHUNK = 128  # output t-chunk (PSUM partition dim)


# ---------------------------------------------------------------- metadata --
class Band:
    def __init__(self, g, i, f0, bw):
        self.g, self.i, self.f0, self.bw = g, i, f0, bw


BANDS = []
_f0 = 0
for _g, (_n, _bw) in enumerate(GROUPS):
    for _i in range(_n):
        BANDS.append(Band(_g, _i, _f0, _bw))
        _f0 += _bw
assert _f0 == F_BINS and len(BANDS) == 31


class Pack:
    def __init__(self, pid, band_ids, qset, quad):
        self.pid = pid
        self.band_ids = list(band_ids)
        self.n = len(self.band_ids)
        self.bws = [BANDS[k].bw for k in self.band_ids]
        self.offs = list(np.cumsum([0] + self.bws[:-1]))  # bin offset in pack
        self.F2 = sum(self.bws)                           # bins in pack
        self.F = 2 * self.F2                              # feature rows
        self.K = self.F + 1                               # + ones row
        self.K32 = (self.K + 31) // 32 * 32               # tile partitions
        self.k0 = self.band_ids[0]                        # first global band
        self.f0 = BANDS[self.k0].f0                       # first freq bin
        self.qset = qset                                  # 'A' or 'B'
        self.quad = quad                                  # PSUM row base /32
        assert self.K <= 128 and self.n <= 32


# matmul tile_position bases are limited to {0, 32, 64} (quadrant-3 HW bug),
# so at most 3 packs share a stats/srstd tile set.
PACKS = [
    Pack(0, range(0, 10), 'A', 0),
    Pack(1, range(10, 16), 'A', 1),
    Pack(2, range(16, 22), 'A', 2),
    Pack(3, range(22, 25), 'B', 0),
    Pack(4, range(25, 28), 'B', 1),
    Pack(5, range(28, 31), 'B', 2),
]
# emission order interleaving the two qsets so the A and B stats chains
# advance in parallel instead of B queueing behind all of A
PACKS_IL = [PACKS[0], PACKS[3], PACKS[1], PACKS[4], PACKS[2], PACKS[5]]
QSETS = "AB"
EREP_COLS = max(p.F for p in PACKS)  # 118

import os as _os

# schedule knobs (env-overridable for offline tuning; defaults are tuned)
SMALL_FIRST = int(_os.environ.get("K_SMALL_FIRST", "0"))
SLAB_POS = _os.environ.get("K_SLAB_POS", "start")  # start | after0
WARMUP = int(_os.environ.get("K_WARMUP", "8"))
DUMMY_SQRT = int(_os.environ.get("K_DUMMY_SQRT", "1"))
XMULT_POOL = int(_os.environ.get("K_XMULT_POOL", "0"))
XMULT_POOL_PIDS = tuple(
    int(c) for c in _os.environ.get("K_XMULT_POOL_PIDS", "135"))

# first spans optionally small so the first output chunk (and with it
# the serialized output-DMA stream) starts as early as possible
if SMALL_FIRST == 1:
    SPANS = [(0, 128), (128, 384)] + \
        [(s0, min(SPAN, T - s0)) for s0 in range(SPAN, T, SPAN)]
elif SMALL_FIRST == 2:
    SPANS = [(0, 256), (256, 256)] + \
        [(s0, min(SPAN, T - s0)) for s0 in range(SPAN, T, SPAN)]
else:
    SPANS = [(s0, min(SPAN, T - s0)) for s0 in range(0, T, SPAN)]
OB_COLS = 31 * C  # 3968


def _band_rows(p, j):
    """SBUF plane-major rows of band j within pack p (re rows, im rows)."""
    o, bw = p.offs[j], p.bws[j]
    return [(o, bw), (p.F2 + o, bw)]


def _blocks(p):
    """<=512-wide column blocks of pack p's (n*C) output."""
    out = []
    for n0 in range(0, p.n * C, 512):
        nw = min(512, p.n * C - n0)
        out.append((n0, nw))
    return out


# drain engine per block index within a chunk (10 blocks):
# widths [512,512,256 | 512,256 | 512,256 | 384 | 384 | 384]
# GPSIMD cannot read PSUM, so drains run on scalar/vector only.
DRAIN_ENG = ["scalar", "scalar", "vector",   # p0
             "scalar", "vector",             # p1
             "scalar", "vector",             # p2
             "scalar",                       # p3
             "vector",                       # p4
             "vector"]                       # p5


def _round_f32r(a):
    """RNE-round fp32 to fp32r (11-bit mantissa; low 12 bits zero) so the
    PE's truncating fp32r datapath sees exactly these values."""
    a = np.ascontiguousarray(np.asarray(a, np.float32))
    u = a.view(np.uint32).copy()
    lsb = (u >> 12) & 1
    u = u + 0x7FF + lsb
    u &= np.uint32(0xFFFFF000)
    return u.view(np.float32)


# ------------------------------------------------------------- host params --
def _host_params(inputs):
    f32 = np.float32
    wextall = np.zeros((128, OB_COLS), f32)
    selall = np.zeros((128, 32 * len(PACKS)), f32)
    erepall = np.zeros((96, 2 * EREP_COLS), f32)
    for p in PACKS:
        qi = QSETS.index(p.qset)
        for j, k in enumerate(p.band_ids):
            b = BANDS[k]
            bw, d = b.bw, 2 * b.bw
            W = np.asarray(inputs[f"g{b.g}_W"][b.i], f32)        # (d, C)
            gam = np.asarray(inputs[f"g{b.g}_gamma"][b.i], f32)  # (d,)
            bet = np.asarray(inputs[f"g{b.g}_beta"][b.i], f32)
            bias = np.asarray(inputs[f"g{b.g}_b"][b.i], f32)     # (C,)
            Wg = gam[:, None] * W
            Wg = Wg - Wg.mean(0, keepdims=True)  # fold -mean(q)*u into Wg
            cols = slice(k * C, (k + 1) * C)
            # device row layout is plane-major: row pl*F2 + off_j + f
            # holds (plane pl, bin f) of band j == torch feature 2f+pl
            for pl in range(2):
                r0 = pl * p.F2 + p.offs[j]
                wextall[r0:r0 + bw, cols] = Wg[2 * np.arange(bw) + pl]
                selall[r0:r0 + bw, 32 * p.pid + j] = 1.0 / d
                erepall[32 * p.quad + j,
                        qi * EREP_COLS + r0:qi * EREP_COLS + r0 + bw] = 1.0
            wextall[p.F, cols] = bias + bet @ W
    return {"wextall": _round_f32r(wextall), "selall": _round_f32r(selall),
            "erepall": erepall}


# ------------------------------------------------------------ device build --
_CACHE = {}


def _build():
    if "nc" in _CACHE:
        return _CACHE["nc"]
    import concourse.bacc as bacc
    import concourse.tile as tile
    from concourse import mybir

    Alu = mybir.AluOpType
    Act = mybir.ActivationFunctionType
    F32 = mybir.dt.float32
    F32R = mybir.dt.float32r

    def r(ap):  # float32r view for PE streaming
        return ap.bitcast(F32R)

    nc = bacc.Bacc("TRN2", target_bir_lowering=False, debug=False, num_devices=8)
    x_d = nc.dram_tensor("x", [2, F_BINS, T], F32, kind="ExternalInput")
    out_d = nc.dram_tensor("out", [31, T, C], F32, kind="ExternalOutput")
    wext_d = nc.dram_tensor("wextall", [128, OB_COLS], F32, kind="ExternalInput")
    selx_d = nc.dram_tensor("selall", [128, 32 * len(PACKS)], F32,
                            kind="ExternalInput")
    erep_d = nc.dram_tensor("erepall", [96, 2 * EREP_COLS], F32,
                            kind="ExternalInput")

    with tile.TileContext(nc) as tc:
        with (
            tc.tile_pool(name="const", bufs=1) as const,
            tc.tile_pool(name="xsqp", bufs=7) as xsqp,
            tc.tile_pool(name="stt", bufs=2) as stt,
            tc.tile_pool(name="obp",
                         bufs=int(_os.environ.get("K_OBP", "3"))) as obp,
            tc.tile_pool(name="srp", bufs=2) as srp,
            tc.tile_pool(name="stps", bufs=1, space="PSUM") as stps,
            tc.tile_pool(name="outps", bufs=4, space="PSUM") as outps,
        ):
            # ---- resident constants: tiny stats params, first x slab, wext.
            # Later x slabs are prefetched inside the span loop (slab s+1
            # issued during span s) so the output DMAs are not queued behind
            # the whole input load on the issuing sequencer.
            selall = const.tile([128, 32 * len(PACKS)], F32, tag="sel", name="selall")
            nc.sync.dma_start(out=r(selall[:]), in_=r(selx_d[:]))
            erepall = const.tile([96, 2 * EREP_COLS], F32, tag="er", name="erepall")
            nc.sync.dma_start(out=r(erepall[:]), in_=r(erep_d[:]))
            xin = {}
            for p in PACKS:
                xin[p.pid] = const.tile([p.F, T], F32, tag=f"xin{p.pid}", name=f"xin{p.pid}")

            SLAB_ENG = _os.environ.get("K_SLAB_ENG", "sync")

            def load_slab(s0, sw):
                # per-plane DMAs: dst keeps a single-level partition dim
                for p in PACKS:
                    for pl in range(2):
                        s_ = x_d[pl, p.f0:p.f0 + p.F2, s0:s0 + sw]
                        d_ = xin[p.pid][pl * p.F2:(pl + 1) * p.F2,
                                        s0:s0 + sw]
                        eng = nc.sync
                        if SLAB_ENG == "mix" and (p.pid + pl) % 2 == 1:
                            eng = nc.scalar
                        eng.dma_start(out=r(d_), in_=r(s_))

            # startup: cover the first SLAB0W spans with one wide slab per
            # (pack, plane) so fewer HWDGE setups serialize before the
            # pipeline fills
            SLAB0W = int(_os.environ.get("K_SLAB0W", "5"))
            _s0w = min(SLAB0W, len(SPANS))
            _w0 = sum(sw for _, sw in SPANS[:_s0w])
            _groups = [int(g) for g in
                       _os.environ.get("K_SLAB0GROUPS", "").split(",") if g]
            if _groups:
                assert sum(_groups) == _w0
                _g0 = 0
                for _gw in _groups:
                    load_slab(_g0, _gw)
                    _g0 += _gw
            else:
                load_slab(0, _w0)
            wext = const.tile([128, OB_COLS], F32, tag="wx", name="wextall")
            _weng = {"sync": nc.sync, "scalar": nc.scalar}[
                _os.environ.get("K_WEXT_ENG", "sync")]
            _weng.dma_start(out=r(wext[:]), in_=r(wext_d[:]))



            def selx(p):
                return selall[0:p.F, 32 * p.pid:32 * p.pid + 32]

            def erep(p):
                qi = QSETS.index(p.qset)
                q0 = 32 * p.quad
                return erepall[q0:q0 + p.n,
                               qi * EREP_COLS:qi * EREP_COLS + p.F]

            eps_t = const.tile([128, 1], F32, tag="epsc", name="epsc")
            nc.vector.memset(eps_t[:], EPS)
            ones_t = const.tile([32, SPAN], F32, tag="ones", name="ones")
            nc.vector.memset(ones_t[:], 1.0)
            # PE p-state warm-up: keep the PE busy during the initial x-slab
            # load so span-0 matmuls run at full clock.  Reads the freshly
            # memset ones tile; write targets rotate through the outps slots
            # and are never read.
            WARM_SRC = _os.environ.get("K_WARM_SRC", "ones")
            for _ in range(WARMUP):
                wm = outps.tile([128, 512], F32, tag="op", name="wm")
                if WARM_SRC == "wext":
                    # depends on the wext load, so the warm-up fires right
                    # before span-0's first matmuls instead of 15us earlier
                    nc.tensor.matmul(
                        wm[0:128, 0:192], r(wext[0:32, 0:128]),
                        r(wext[0:32, 0:192]), start=True, stop=True)
                else:
                    nc.tensor.matmul(
                        wm[0:128, 0:192], r(ones_t[0:32, 0:128]),
                        r(ones_t[0:32, 0:192]), start=True, stop=True)
            if DUMMY_SQRT:
                # dummy Sqrt: makes the compiler pick the sqrt-bearing
                # activation table up front (it also holds Square/Copy), so
                # no table reload lands on the span-0 critical chain.
                scr0 = const.tile([1, 1], F32, tag="scr0", name="scr0")
                nc.scalar.activation(scr0[0:1, 0:1], eps_t[0:1, 0:1],
                                     Act.Sqrt, bias=eps_t[0:1, 0:1],
                                     scale=1.0)

            # persistent double-buffered lhsT tiles; the ones row (row F)
            # is written once, q rows get rewritten every span.
            # engine partition writes must start 32-aligned, so the ones
            # row (row F) is filled by a 32-row aligned copy; the x rows it
            # clobbers are rewritten by the per-span q prep before use.
            xpt = {}
            for i, p in enumerate(PACKS):
                for par in range(2):
                    t_ = const.tile([p.K32, SPAN], F32, tag=f"xp{p.pid}_{par}",
                                    name=f"xp{p.pid}_{par}")
                    xpt[(p.pid, par)] = t_
                    for m0 in range(p.F // 32 * 32, p.K32, 32):
                        if (i + par) % 2 == 0:
                            nc.vector.tensor_copy(r(t_[m0:m0 + 32, :]),
                                                  ones_t[0:32, :])
                        else:
                            nc.scalar.activation(r(t_[m0:m0 + 32, :]),
                                                 ones_t[0:32, :], Act.Copy)

            from concourse.dve_ops import (
                RECIP_APPROX_FAST_CONSTS as _RC,
                RECIPROCAL_APPROX_FAST as _RF,
            )
            srstd_by_si = {}

            XSQ0 = _os.environ.get("K_XSQ0", "pool")

            def emit_stats(si):
                """Stats + rstd chain for span si.  Called one span AHEAD
                (software pipelining) so srstd is ready when the span's
                prep/mains start.  fp32r matmuls may only write PSUM at
                partition base 0, so per-band sums land in per-pack [0:32]
                tiles; the partition-shifting Square/copy then rebuilds the
                quadrant layout in SBUF so the rsqrt tail stays batched."""
                s0, sw = SPANS[si]
                srstd = {qs: srp.tile([96, SPAN], F32, tag=f"sr{qs}",
                                      name=f"sr{qs}") for qs in QSETS}
                srstd_by_si[si] = srstd
                musq = {qs: stt.tile([96, SPAN], F32, tag=f"musq{qs}",
                                     name=f"musq{qs}") for qs in QSETS}
                msqs = {qs: stt.tile([96, SPAN], F32, tag=f"msqs{qs}",
                                     name=f"msqs{qs}") for qs in QSETS}
                for p in PACKS:
                    q0 = 32 * p.quad
                    xin_f = xin[p.pid][:, s0:s0 + sw]
                    xsq = xsqp.tile([128, SPAN], F32, tag="xsq", name="xsq")
                    # x^2 on GPSIMD (SBUF->SBUF is legal there), freeing the
                    # scalar engine for PSUM drains; span 0 spreads it over
                    # the then-idle Act/DVE engines to compress the first
                    # span's critical chain
                    if si == 0 and XSQ0 == "mix":
                        if p.pid % 2 == 0:
                            nc.scalar.activation(
                                xsq[0:p.F, :sw], xin_f, Act.Square)
                        else:
                            nc.vector.tensor_tensor(
                                xsq[0:p.F, :sw], xin_f, xin_f,
                                op=Alu.mult)
                    else:
                        # fp32 out: the Q7 software engine has no fp32r
                        # output path; the msq matmul reads it as fp32
                        nc.gpsimd.tensor_tensor(
                            xsq[0:p.F, :sw], xin_f, xin_f, op=Alu.mult)
                    mu_p = stps.tile([32, SPAN], F32, tag="mu", name="mu",
                                     bufs=2)
                    msq_p = stps.tile([32, SPAN], F32, tag="ms", name="ms",
                                      bufs=2)
                    nc.tensor.matmul(
                        mu_p[0:32, :sw], r(selx(p)),
                        r(xin_f), start=True, stop=True)
                    nc.tensor.matmul(
                        msq_p[0:32, :sw], selx(p),
                        xsq[0:p.F, :sw], start=True, stop=True)
                    # partition-shifting ops rebuild the quadrant layout
                    nc.scalar.activation(
                        musq[p.qset][q0:q0 + 32, :sw], mu_p[0:32, :sw],
                        Act.Square)
                    nc.vector.tensor_copy(
                        msqs[p.qset][q0:q0 + 32, :sw], msq_p[0:32, :sw])
                # ---- B) rstd = rsqrt(msq - mu^2 + eps), batched per set
                for qs in QSETS:
                    var = stt.tile([96, SPAN], F32, tag="var", name="var")
                    nc.vector.tensor_tensor(
                        var[0:96, :sw], msqs[qs][0:96, :sw],
                        musq[qs][0:96, :sw], op=Alu.subtract)
                    sq = stt.tile([96, SPAN], F32, tag="sq", name="sq")
                    nc.scalar.activation(
                        sq[0:96, :sw], var[0:96, :sw], Act.Sqrt,
                        bias=eps_t[0:96, 0:1], scale=1.0)
                    # reciprocal_approx_fast with an fp32r-rounded output
                    # view (its wrapper asserts fp32, so call the custom-DVE
                    # op directly; the seed trick only needs fp32 INPUT bits)
                    nc.vector._custom_dve(
                        _RF, out=r(srstd[qs][0:96, :sw]),
                        in0=sq[0:96, :sw], s0=_RC["s0"], s1=_RC["s1"],
                        imm2=_RC["imm2"])

            emit_stats(0)
            for si, (s0, sw) in enumerate(SPANS):
                par = si % 2
                if SLAB_POS == "start" and _s0w <= si + 1 < len(SPANS):
                    load_slab(*SPANS[si + 1])
                srstd = srstd_by_si.pop(si)
                # ---- C) pack lhsT prep: q = x * rstd_rep
                for p in PACKS:
                    q0 = 32 * p.quad
                    t_ = xpt[(p.pid, par)]
                    rr = outps.tile([128, 512], F32, tag="op", name="rr")
                    nc.tensor.matmul(
                        rr[0:p.F, :sw],
                        r(erep(p)),
                        r(srstd[p.qset][q0:q0 + p.n, :sw]),
                        start=True, stop=True)
                    xm_eng = (nc.gpsimd if XMULT_POOL and
                              p.pid in XMULT_POOL_PIDS else nc.vector)
                    xm_eng.tensor_tensor(
                        r(t_[0:p.F, :sw]), xin[p.pid][:, s0:s0 + sw],
                        rr[0:p.F, :sw], op=Alu.mult)
                # ---- D) main matmuls + drain + one DMA per t-chunk
                for c0 in range(s0, s0 + sw, CHUNK):
                    cw = min(CHUNK, s0 + sw - c0)
                    ob = obp.tile([128, OB_COLS], F32, tag="ob", name="ob")
                    di = 0
                    for p in PACKS:
                        lhsT = xpt[(p.pid, par)][0:p.K, c0 - s0:c0 - s0 + cw]
                        obase = p.k0 * C
                        for (n0, nw) in _blocks(p):
                            op = outps.tile([128, 512], F32, tag="op", name="op")
                            nc.tensor.matmul(
                                op[0:cw, 0:nw], r(lhsT),
                                r(wext[0:p.K, obase + n0:obase + n0 + nw]),
                                start=True, stop=True)
                            eng = DRAIN_ENG[di]
                            dst = ob[0:cw, obase + n0:obase + n0 + nw]
                            if eng == "scalar":
                                nc.scalar.activation(dst, op[0:cw, 0:nw],
                                                     Act.Copy)
                            elif eng == "vector":
                                nc.vector.tensor_copy(dst, op[0:cw, 0:nw])
                            else:
                                nc.gpsimd.tensor_copy(dst, op[0:cw, 0:nw])
                            di += 1
                    if si == 0 and int(_os.environ.get("K_SPLIT0", "0")):
                        # span 0 only: ship A-pack columns as soon as their
                        # drains land, B-pack columns separately
                        dst = out_d[0:22, c0:c0 + cw, :].rearrange(
                            "j t c -> t j c")
                        srch = ob[0:cw, 0:22 * C].rearrange(
                            "t (j c) -> t j c", c=C)
                        nc.sync.dma_start(out=dst, in_=srch)
                        dst = out_d[22:31, c0:c0 + cw, :].rearrange(
                            "j t c -> t j c")
                        srch = ob[0:cw, 22 * C:].rearrange(
                            "t (j c) -> t j c", c=C)
                        nc.sync.dma_start(out=dst, in_=srch)
                    else:
                        dst = out_d[:, c0:c0 + cw, :].rearrange(
                            "j t c -> t j c")
                        src = ob[0:cw, :].rearrange("t (j c) -> t j c", c=C)
                        nc.sync.dma_start(out=dst, in_=src)
                    # prefetch next span's x columns right after this span's
                    # first output DMA: SP has issued the head of the output
                    # stream, and the slab lands well before span s+1 needs it
                    if SLAB_POS == "after0" and c0 == s0 and si + 1 < len(SPANS):
                        load_slab(*SPANS[si + 1])
                if si + 1 < len(SPANS):
                    emit_stats(si + 1)

    nc.compile()
    _CACHE["nc"] = nc
    return nc


# ------------------------------------------------------------------ driver --
def kernel(**inputs):
    from concourse.bass_utils import run_bass_kernel_spmd

    x = _round_f32r(np.asarray(inputs["x"], np.float32))
    B = x.shape[0]
    assert x.shape == (8, 2, F_BINS, T)
    ext = _host_params(inputs)
    nc = _build()
    in_maps = []
    for b in range(B):
        m = {"x": x[b]}
        m.update(ext)
        in_maps.append(m)
    res = run_bass_kernel_spmd(nc, in_maps, core_ids=list(range(8)))
    out = np.stack([res.results[b]["out"] for b in range(B)], axis=0)
    return out.astype(np.float32, copy=False)



# revision 20
# speedup vs baseline: 1.1491x; 1.1491x over previous
"""BandSplit (BSRNN) Trainium2 kernel, fp16 edition.

Math per band k (31 bands over 257 freq bins, band widths 3/6/16/27):
  xg = x[b, :, band_bins, t] flattened to d = 2*bw features (torch order:
       bin-major, re/im minor)
  out[b, k, t, :] = LayerNorm_d(xg) @ W_k + b_k          (d -> C=128)

Algebraic refactor (per band, per t), with q = x * rstd:
  out = q @ (Wg - colmean_d(Wg)) + bb
  with host-precomputed  Wg = gamma*W,  bb = b + beta @ W.

fp16 strategy: the kernel is DMA-bound (output is 31*3000*128 floats per
core), so x / params / staging / output all move as fp16 (quantization
error ~1e-3 rel, well inside the 2e-2 gate and below the old fp32r
truncation error).  The output DRAM layout is [T, 31, C] so each DMA
descriptor is one t-row of 31*128 contiguous fp16 = 7936 B (>=512 B keeps
the DMA bus at full rate); the host transposes back to [31, T, C] fp32.
fp16 matmuls run 1 cycle/row at any width and may write PSUM at 32-aligned
quadrant bases, so per-band stats land directly in quadrant layout (no
partition-shift copies).

Sharding: batch-parallel, core b handles x[b] (B=8 = n_cores).
"""

import numpy as np

T = 3000
C = 128
F_BINS = 257
EPS = 1e-5
GROUPS = [(10, 3), (12, 6), (8, 16), (1, 27)]  # (n_bands, bins_per_band)

SPAN = 512   # stats/prep span (free dim of PSUM bank) == x-slab width
CHUNK = 128  # output t-chunk (PSUM partition dim)


# ---------------------------------------------------------------- metadata --
class Band:
    def __init__(self, g, i, f0, bw):
        self.g, self.i, self.f0, self.bw = g, i, f0, bw


BANDS = []
_f0 = 0
for _g, (_n, _bw) in enumerate(GROUPS):
    for _i in range(_n):
        BANDS.append(Band(_g, _i, _f0, _bw))
        _f0 += _bw
assert _f0 == F_BINS and len(BANDS) == 31


class Pack:
    def __init__(self, pid, band_ids, qset, quad):
        self.pid = pid
        self.band_ids = list(band_ids)
        self.n = len(self.band_ids)
        self.bws = [BANDS[k].bw for k in self.band_ids]
        self.offs = list(np.cumsum([0] + self.bws[:-1]))  # bin offset in pack
        self.F2 = sum(self.bws)                           # bins in pack
        self.F = 2 * self.F2                              # feature rows
        self.K = self.F + 1                               # + ones row
        self.K32 = (self.K + 31) // 32 * 32               # tile partitions
        self.k0 = self.band_ids[0]                        # first global band
        self.f0 = BANDS[self.k0].f0                       # first freq bin
        self.qset = qset                                  # 'A' or 'B'
        self.quad = quad                                  # PSUM row base /32
        assert self.K <= 128 and self.n <= 32


# matmul tile_position bases are limited to {0, 32, 64} (quadrant-3 HW bug),
# so at most 3 packs share a stats/srstd tile set.
PACKS = [
    Pack(0, range(0, 10), 'A', 0),
    Pack(1, range(10, 16), 'A', 1),
    Pack(2, range(16, 22), 'A', 2),
    Pack(3, range(22, 25), 'B', 0),
    Pack(4, range(25, 28), 'B', 1),
    Pack(5, range(28, 31), 'B', 2),
]
# emission order interleaving the two qsets so the A and B stats chains
# advance in parallel instead of B queueing behind all of A
PACKS_IL = [PACKS[0], PACKS[3], PACKS[1], PACKS[4], PACKS[2], PACKS[5]]
QSETS = "AB"
EREP_COLS = max(p.F for p in PACKS)  # 118

import os as _os

# schedule knobs (env-overridable for offline tuning; defaults are tuned)
WARMUP = int(_os.environ.get("K_WARMUP", "8"))
DUMMY_SQRT = int(_os.environ.get("K_DUMMY_SQRT", "1"))
SLAB0W = _os.environ.get("K_SLAB0GROUPS", "512,2488")

SPANS = [(s0, min(SPAN, T - s0)) for s0 in range(0, T, SPAN)]
OB_COLS = 31 * C  # 3968


def _band_rows(p, j):
    """SBUF plane-major rows of band j within pack p (re rows, im rows)."""
    o, bw = p.offs[j], p.bws[j]
    return [(o, bw), (p.F2 + o, bw)]


def _blocks(p):
    """<=512-wide column blocks of pack p's (n*C) output."""
    out = []
    for n0 in range(0, p.n * C, 512):
        nw = min(512, p.n * C - n0)
        out.append((n0, nw))
    return out


# drain engine per block index within a chunk (10 blocks):
# widths [512,512,256 | 512,256 | 512,256 | 384 | 384 | 384]
# GPSIMD cannot read PSUM, so drains run on scalar/vector only.  Balanced
# for fp16: Act also carries Square+Sqrt (2048 cyc/span), DVE carries
# var+recip+xmult (5120 cyc/span); Act=2688, DVE=1280 drain cols/chunk.
DRAIN_ENG = ["scalar", "scalar", "scalar",   # p0
             "scalar", "vector",             # p1
             "scalar", "vector",             # p2
             "scalar",                       # p3
             "vector",                       # p4
             "vector"]                       # p5


def _round_f32r(a):
    """RNE-round fp32 to fp32r (11-bit mantissa; low 12 bits zero) so the
    PE's truncating fp32r datapath sees exactly these values."""
    a = np.ascontiguousarray(np.asarray(a, np.float32))
    u = a.view(np.uint32).copy()
    lsb = (u >> 12) & 1
    u = u + 0x7FF + lsb
    u &= np.uint32(0xFFFFF000)
    return u.view(np.float32)


# ------------------------------------------------------------- host params --
def _host_params(inputs):
    f32 = np.float32
    wextall = np.zeros((128, OB_COLS), f32)
    selall = np.zeros((128, 32 * len(PACKS)), f32)
    erepall = np.zeros((96, 2 * EREP_COLS), f32)
    for p in PACKS:
        qi = QSETS.index(p.qset)
        for j, k in enumerate(p.band_ids):
            b = BANDS[k]
            bw, d = b.bw, 2 * b.bw
            W = np.asarray(inputs[f"g{b.g}_W"][b.i], f32)        # (d, C)
            gam = np.asarray(inputs[f"g{b.g}_gamma"][b.i], f32)  # (d,)
            bet = np.asarray(inputs[f"g{b.g}_beta"][b.i], f32)
            bias = np.asarray(inputs[f"g{b.g}_b"][b.i], f32)     # (C,)
            Wg = gam[:, None] * W
            Wg = Wg - Wg.mean(0, keepdims=True)  # fold -mean(q)*u into Wg
            cols = slice(k * C, (k + 1) * C)
            # device row layout is plane-major: row pl*F2 + off_j + f
            # holds (plane pl, bin f) of band j == torch feature 2f+pl
            for pl in range(2):
                r0 = pl * p.F2 + p.offs[j]
                wextall[r0:r0 + bw, cols] = Wg[2 * np.arange(bw) + pl]
                selall[r0:r0 + bw, 32 * p.pid + j] = 1.0 / d
                erepall[32 * p.quad + j,
                        qi * EREP_COLS + r0:qi * EREP_COLS + r0 + bw] = 1.0
            wextall[p.F, cols] = bias + bet @ W
    return {"wextall": wextall.astype(np.float16),
            "selall": selall.astype(np.float16),
            "erepall": erepall.astype(np.float16)}


# ------------------------------------------------------------ device build --
_CACHE = {}


def _build():
    if "nc" in _CACHE:
        return _CACHE["nc"]
    import concourse.bacc as bacc
    import concourse.tile as tile
    from concourse import mybir

    Alu = mybir.AluOpType
    Act = mybir.ActivationFunctionType
    F32 = mybir.dt.float32
    F32R = mybir.dt.float32r
    F16 = mybir.dt.float16

    def r(ap):  # float32r view for PE streaming of fp32 data
        return ap.bitcast(F32R)

    DBG = int(_os.environ.get("K_DEBUG_SRSTD", "0"))
    nc = bacc.Bacc("TRN2", target_bir_lowering=False, debug=False, num_devices=8)
    x_d = nc.dram_tensor("x", [2, F_BINS, T], F16, kind="ExternalInput")
    out_d = nc.dram_tensor("out", [T, 31, C], F16, kind="ExternalOutput")
    dbg_d = None
    if DBG:
        dbg_d = nc.dram_tensor("dbg", [6, 96, SPAN], F32, kind="ExternalOutput")
    DBGQ = int(_os.environ.get("K_DEBUG_Q", "0"))
    dbgq_d = None
    if DBGQ:
        dbgq_d = nc.dram_tensor("dbgq", [6, 128, SPAN], F16,
                                kind="ExternalOutput")
    wext_d = nc.dram_tensor("wextall", [128, OB_COLS], F16, kind="ExternalInput")
    selx_d = nc.dram_tensor("selall", [128, 32 * len(PACKS)], F16,
                            kind="ExternalInput")
    erep_d = nc.dram_tensor("erepall", [96, 2 * EREP_COLS], F16,
                            kind="ExternalInput")

    with tile.TileContext(nc) as tc:
        with (
            tc.tile_pool(name="const", bufs=1) as const,
            tc.tile_pool(name="xsqp", bufs=7) as xsqp,
            tc.tile_pool(name="stt", bufs=2) as stt,
            tc.tile_pool(name="obp",
                         bufs=int(_os.environ.get("K_OBP", "3"))) as obp,
            tc.tile_pool(name="srp", bufs=2) as srp,
            tc.tile_pool(name="stps", bufs=1, space="PSUM") as stps,
            tc.tile_pool(name="outps", bufs=4, space="PSUM") as outps,
        ):
            # ---- resident constants: tiny stats params, first x slab, wext.
            selall = const.tile([128, 32 * len(PACKS)], F16, tag="sel", name="selall")
            nc.sync.dma_start(out=selall[:], in_=selx_d[:])
            erepall = const.tile([96, 2 * EREP_COLS], F16, tag="er", name="erepall")
            nc.sync.dma_start(out=erepall[:], in_=erep_d[:])
            xin = {}
            for p in PACKS:
                xin[p.pid] = const.tile([p.F, T], F16, tag=f"xin{p.pid}",
                                        name=f"xin{p.pid}")

            SLAB3D = int(_os.environ.get("K_SLAB3D", "0"))

            def load_slab(s0, sw):
                for p in PACKS:
                    if SLAB3D:
                        # one DMA per pack covering both planes: src is a 3D
                        # [2, F2, sw] DRAM AP, dst a plane-major [F, sw] AP
                        s_ = x_d[:, p.f0:p.f0 + p.F2, s0:s0 + sw]
                        d_ = xin[p.pid][0:p.F, s0:s0 + sw]
                        nc.sync.dma_start(out=d_, in_=s_)
                    else:
                        for pl in range(2):
                            s_ = x_d[pl, p.f0:p.f0 + p.F2, s0:s0 + sw]
                            d_ = xin[p.pid][pl * p.F2:(pl + 1) * p.F2,
                                            s0:s0 + sw]
                            nc.sync.dma_start(out=d_, in_=s_)

            # startup: first group covers span 0 only so stats can start
            # ASAP; the rest of the head slabs follow in a second DMA wave
            _groups = [int(g) for g in SLAB0W.split(",") if g]
            assert sum(_groups) <= T
            _g0 = 0
            for _gw in _groups:
                load_slab(_g0, _gw)
                _g0 += _gw
            _s0w = 0  # spans fully covered by startup groups
            _acc = 0
            for _s0, _sw in SPANS:
                if _acc + _sw <= _g0:
                    _acc += _sw
                    _s0w += 1
                else:
                    break
            wext = const.tile([128, OB_COLS], F16, tag="wx", name="wextall")
            nc.sync.dma_start(out=wext[:], in_=wext_d[:])

            def selx(p):
                return selall[0:p.F, 32 * p.pid:32 * p.pid + 32]

            def erep(p):
                qi = QSETS.index(p.qset)
                q0 = 32 * p.quad
                return erepall[q0:q0 + p.n,
                               qi * EREP_COLS:qi * EREP_COLS + p.F]

            eps_t = const.tile([128, 1], F32, tag="epsc", name="epsc")
            nc.vector.memset(eps_t[:], EPS)
            ones_t = const.tile([32, SPAN], F16, tag="ones", name="ones")
            nc.vector.memset(ones_t[:], 1.0)
            # PE p-state warm-up: keep the PE busy during the initial x-slab
            # load so span-0 matmuls run at full clock.
            for _ in range(WARMUP):
                wm = outps.tile([128, 512], F32, tag="op", name="wm")
                nc.tensor.matmul(
                    wm[0:128, 0:192], ones_t[0:32, 0:128],
                    ones_t[0:32, 0:192], start=True, stop=True)
            if DUMMY_SQRT:
                # dummy Sqrt: makes the compiler pick the sqrt-bearing
                # activation table up front (it also holds Square/Copy), so
                # no table reload lands on the span-0 critical chain.
                scr0 = const.tile([1, 1], F32, tag="scr0", name="scr0")
                nc.scalar.activation(scr0[0:1, 0:1], eps_t[0:1, 0:1],
                                     Act.Sqrt, bias=eps_t[0:1, 0:1],
                                     scale=1.0)

            # persistent double-buffered lhsT tiles; the ones row (row F)
            # is written once, q rows get rewritten every span.
            # engine partition writes must start 32-aligned, so the ones
            # row (row F) is filled by a 32-row aligned copy; the x rows it
            # clobbers are rewritten by the per-span q prep before use.
            xpt = {}
            for i, p in enumerate(PACKS):
                for par in range(2):
                    t_ = const.tile([p.K32, SPAN], F16, tag=f"xp{p.pid}_{par}",
                                    name=f"xp{p.pid}_{par}")
                    xpt[(p.pid, par)] = t_
                    for m0 in range(p.F // 32 * 32, p.K32, 32):
                        if (i + par) % 2 == 0:
                            nc.vector.tensor_copy(t_[m0:m0 + 32, :],
                                                  ones_t[0:32, :])
                        else:
                            nc.scalar.activation(t_[m0:m0 + 32, :],
                                                 ones_t[0:32, :], Act.Copy)

            from concourse.dve_ops import (
                RECIP_APPROX_FAST_CONSTS as _RC,
                RECIPROCAL_APPROX_FAST as _RF,
            )
            srstd_by_si = {}

            def emit_stats(si):
                """Stats + rstd chain for span si.  Called one span AHEAD
                (software pipelining) so srstd is ready when the span's
                prep/mains start.  fp16 matmuls may write PSUM at 32-aligned
                bases, so per-pack sums land directly in the quadrant rows
                of per-qset [96, SPAN] PSUM tiles."""
                s0, sw = SPANS[si]
                srstd = {qs: srp.tile([96, SPAN], F32, tag=f"sr{qs}",
                                      name=f"sr{qs}") for qs in QSETS}
                srstd16 = {qs: srp.tile([96, SPAN], F16, tag=f"sr16{qs}",
                                        name=f"sr16{qs}") for qs in QSETS}
                srstd_by_si[si] = srstd16
                mu_ps = {qs: stps.tile([96, SPAN], F32, tag=f"mu{qs}",
                                       name=f"mu{qs}") for qs in QSETS}
                msq_ps = {qs: stps.tile([96, SPAN], F32, tag=f"ms{qs}",
                                        name=f"ms{qs}") for qs in QSETS}
                for p in PACKS_IL:
                    q0 = 32 * p.quad
                    xin_f = xin[p.pid][:, s0:s0 + sw]
                    # x^2 on GPSIMD (SBUF->SBUF is legal there), freeing the
                    # scalar engine for PSUM drains
                    xsq = xsqp.tile([128, SPAN], F16, tag="xsq", name="xsq")
                    nc.gpsimd.tensor_tensor(
                        xsq[0:p.F, :sw], xin_f, xin_f, op=Alu.mult)
                    nc.tensor.matmul(
                        mu_ps[p.qset][q0:q0 + 32, :sw], selx(p),
                        xin_f, start=True, stop=True)
                    nc.tensor.matmul(
                        msq_ps[p.qset][q0:q0 + 32, :sw], selx(p),
                        xsq[0:p.F, :sw], start=True, stop=True)
                # ---- B) rstd = rsqrt(msq - mu^2 + eps), batched per set
                for qs in QSETS:
                    musq = stt.tile([96, SPAN], F32, tag=f"musq{qs}",
                                    name=f"musq{qs}")
                    nc.scalar.activation(
                        musq[0:96, :sw], mu_ps[qs][0:96, :sw], Act.Square)
                    var = stt.tile([96, SPAN], F32, tag="var", name="var")
                    nc.vector.tensor_tensor(
                        var[0:96, :sw], msq_ps[qs][0:96, :sw],
                        musq[0:96, :sw], op=Alu.subtract)
                    sq = stt.tile([96, SPAN], F32, tag="sq", name="sq")
                    nc.scalar.activation(
                        sq[0:96, :sw], var[0:96, :sw], Act.Sqrt,
                        bias=eps_t[0:96, 0:1], scale=1.0)
                    # reciprocal_approx_fast with an fp32r-rounded output
                    # view (its wrapper asserts fp32, so call the custom-DVE
                    # op directly; the seed trick only needs fp32 INPUT bits)
                    nc.vector._custom_dve(
                        _RF, out=srstd[qs][0:96, :sw],
                        in0=sq[0:96, :sw], s0=_RC["s0"], s1=_RC["s1"],
                        imm2=_RC["imm2"])
                    # fp16 copy for the (all-fp16) erep broadcast matmul;
                    # on GPSIMD to keep Act/DVE free for drains
                    nc.gpsimd.tensor_copy(srstd16[qs][0:96, :sw],
                                          srstd[qs][0:96, :sw])
                    if DBG and si == 0:
                        qi = QSETS.index(qs)
                        # dump mu^2, var, srstd for span 0
                        nc.sync.dma_start(out=dbg_d[3 * qi + 0, :, :sw],
                                          in_=musq[0:96, :sw])
                        nc.sync.dma_start(out=dbg_d[3 * qi + 1, :, :sw],
                                          in_=var[0:96, :sw])
                        nc.sync.dma_start(out=dbg_d[3 * qi + 2, :, :sw],
                                          in_=srstd[qs][0:96, :sw])

            emit_stats(0)
            for si, (s0, sw) in enumerate(SPANS):
                par = si % 2
                if _s0w <= si + 1 < len(SPANS):
                    load_slab(*SPANS[si + 1])
                srstd = srstd_by_si.pop(si)
                # ---- C) pack lhsT prep: q = x * rstd_rep
                for p in PACKS_IL:
                    q0 = 32 * p.quad
                    t_ = xpt[(p.pid, par)]
                    rr = outps.tile([128, 512], F32, tag="op", name="rr")
                    nc.tensor.matmul(
                        rr[0:p.F, :sw],
                        erep(p),
                        srstd[p.qset][q0:q0 + p.n, :sw],
                        start=True, stop=True)
                    nc.vector.tensor_tensor(
                        t_[0:p.F, :sw], xin[p.pid][:, s0:s0 + sw],
                        rr[0:p.F, :sw], op=Alu.mult)
                    if DBGQ and si == 0:
                        nc.sync.dma_start(out=dbgq_d[p.pid, 0:p.K32, :sw],
                                          in_=t_[0:p.K32, :sw])
                # ---- D) main matmuls + drain + one DMA per t-chunk
                for c0 in range(s0, s0 + sw, CHUNK):
                    cw = min(CHUNK, s0 + sw - c0)
                    ob = obp.tile([128, OB_COLS], F16, tag="ob", name="ob")
                    di = 0
                    for p in PACKS:
                        lhsT = xpt[(p.pid, par)][0:p.K, c0 - s0:c0 - s0 + cw]
                        obase = p.k0 * C
                        for (n0, nw) in _blocks(p):
                            op = outps.tile([128, 512], F32, tag="op", name="op")
                            nc.tensor.matmul(
                                op[0:cw, 0:nw], lhsT,
                                wext[0:p.K, obase + n0:obase + n0 + nw],
                                start=True, stop=True)
                            eng = DRAIN_ENG[di]
                            dst = ob[0:cw, obase + n0:obase + n0 + nw]
                            if eng == "scalar":
                                nc.scalar.activation(dst, op[0:cw, 0:nw],
                                                     Act.Copy)
                            else:
                                nc.vector.tensor_copy(dst, op[0:cw, 0:nw])
                            di += 1
                    nc.sync.dma_start(
                        out=out_d[c0:c0 + cw, :, :].rearrange(
                            "t j c -> t (j c)"),
                        in_=ob[0:cw, :])
                if si + 1 < len(SPANS):
                    emit_stats(si + 1)

    nc.compile()
    _CACHE["nc"] = nc
    return nc


# ------------------------------------------------------------------ driver --
def kernel(**inputs):
    from concourse.bass_utils import run_bass_kernel_spmd

    x = np.asarray(inputs["x"]).astype(np.float16)
    B = x.shape[0]
    assert x.shape == (8, 2, F_BINS, T)
    ext = _host_params(inputs)
    nc = _build()
    in_maps = []
    for b in range(B):
        m = {"x": x[b]}
        m.update(ext)
        in_maps.append(m)
    res = run_bass_kernel_spmd(nc, in_maps, core_ids=list(range(8)))
    out = np.stack([res.results[b]["out"].transpose(1, 0, 2)
                    for b in range(B)], axis=0)
    return out.astype(np.float32)


# revision 23
# speedup vs baseline: 1.2462x; 1.0844x over previous
"""BandSplit (BSRNN) Trainium2 kernel, fp16 edition.

Math per band k (31 bands over 257 freq bins, band widths 3/6/16/27):
  xg = x[b, :, band_bins, t] flattened to d = 2*bw features (torch order:
       bin-major, re/im minor)
  out[b, k, t, :] = LayerNorm_d(xg) @ W_k + b_k          (d -> C=128)

Algebraic refactor (per band, per t), with q = x * rstd:
  out = q @ (Wg - colmean_d(Wg)) + bb
  with host-precomputed  Wg = gamma*W,  bb = b + beta @ W.

fp16 strategy: the kernel is DMA-bound (output is 31*3000*128 floats per
core), so x / params / staging / output all move as fp16 (quantization
error ~1e-3 rel, well inside the 2e-2 gate and below the old fp32r
truncation error).  The output DRAM layout is [T, 31, C] so each DMA
descriptor is one t-row of 31*128 contiguous fp16 = 7936 B (>=512 B keeps
the DMA bus at full rate); the host transposes back to [31, T, C] fp32.
fp16 matmuls run 1 cycle/row at any width and may write PSUM at 32-aligned
quadrant bases, so per-band stats land directly in quadrant layout (no
partition-shift copies).

Sharding: batch-parallel, core b handles x[b] (B=8 = n_cores).
"""

import numpy as np

T = 3000
C = 128
F_BINS = 257
EPS = 1e-5
GROUPS = [(10, 3), (12, 6), (8, 16), (1, 27)]  # (n_bands, bins_per_band)

SPAN = 512   # stats/prep span (free dim of PSUM bank) == x-slab width
CHUNK = 128  # output t-chunk (PSUM partition dim)


# ---------------------------------------------------------------- metadata --
class Band:
    def __init__(self, g, i, f0, bw):
        self.g, self.i, self.f0, self.bw = g, i, f0, bw


BANDS = []
_f0 = 0
for _g, (_n, _bw) in enumerate(GROUPS):
    for _i in range(_n):
        BANDS.append(Band(_g, _i, _f0, _bw))
        _f0 += _bw
assert _f0 == F_BINS and len(BANDS) == 31


class Pack:
    def __init__(self, pid, band_ids, qset, quad):
        self.pid = pid
        self.band_ids = list(band_ids)
        self.n = len(self.band_ids)
        self.bws = [BANDS[k].bw for k in self.band_ids]
        self.offs = list(np.cumsum([0] + self.bws[:-1]))  # bin offset in pack
        self.F2 = sum(self.bws)                           # bins in pack
        self.F = 2 * self.F2                              # feature rows
        self.K = self.F + 1                               # + ones row
        self.K32 = (self.K + 31) // 32 * 32               # tile partitions
        self.k0 = self.band_ids[0]                        # first global band
        self.f0 = BANDS[self.k0].f0                       # first freq bin
        self.qset = qset                                  # 'A' or 'B'
        self.quad = quad                                  # PSUM row base /32
        assert self.K <= 128 and self.n <= 32


# matmul tile_position bases are limited to {0, 32, 64} (quadrant-3 HW bug),
# so at most 3 packs share a stats/srstd tile set.
PACKS = [
    Pack(0, range(0, 10), 'A', 0),
    Pack(1, range(10, 16), 'A', 1),
    Pack(2, range(16, 22), 'A', 2),
    Pack(3, range(22, 25), 'B', 0),
    Pack(4, range(25, 28), 'B', 1),
    Pack(5, range(28, 31), 'B', 2),
]
# emission order interleaving the two qsets so the A and B stats chains
# advance in parallel instead of B queueing behind all of A
PACKS_IL = [PACKS[0], PACKS[3], PACKS[1], PACKS[4], PACKS[2], PACKS[5]]
QSETS = "AB"
EREP_COLS = max(p.F for p in PACKS)  # 118

import os as _os

# schedule knobs (env-overridable for offline tuning; defaults are tuned)
WARMUP = int(_os.environ.get("K_WARMUP", "8"))
DUMMY_SQRT = int(_os.environ.get("K_DUMMY_SQRT", "1"))
SLAB0W = _os.environ.get("K_SLAB0GROUPS", "512,512")

SPANS = [(s0, min(SPAN, T - s0)) for s0 in range(0, T, SPAN)]
OB_COLS = 31 * C  # 3968


def _band_rows(p, j):
    """SBUF plane-major rows of band j within pack p (re rows, im rows)."""
    o, bw = p.offs[j], p.bws[j]
    return [(o, bw), (p.F2 + o, bw)]


def _blocks(p):
    """<=512-wide column blocks of pack p's (n*C) output."""
    out = []
    for n0 in range(0, p.n * C, 512):
        nw = min(512, p.n * C - n0)
        out.append((n0, nw))
    return out


# drain engine per block index within a chunk (10 blocks):
# widths [512,512,256 | 512,256 | 512,256 | 384 | 384 | 384]
# GPSIMD cannot read PSUM, so drains run on scalar/vector only.  Balanced
# for fp16: Act also carries Square+Sqrt (2048 cyc/span), DVE carries
# var+recip+xmult (5120 cyc/span); Act=2688, DVE=1280 drain cols/chunk.
DRAIN_ENG = ["scalar", "scalar", "scalar",   # p0
             "scalar", "vector",             # p1
             "scalar", "vector",             # p2
             "scalar",                       # p3
             "vector",                       # p4
             "vector"]                       # p5


def _round_f32r(a):
    """RNE-round fp32 to fp32r (11-bit mantissa; low 12 bits zero) so the
    PE's truncating fp32r datapath sees exactly these values."""
    a = np.ascontiguousarray(np.asarray(a, np.float32))
    u = a.view(np.uint32).copy()
    lsb = (u >> 12) & 1
    u = u + 0x7FF + lsb
    u &= np.uint32(0xFFFFF000)
    return u.view(np.float32)


# ------------------------------------------------------------- host params --
def _host_params(inputs):
    f32 = np.float32
    wextall = np.zeros((128, OB_COLS), f32)
    selall = np.zeros((128, 32 * len(PACKS)), f32)
    erepall = np.zeros((96, 2 * EREP_COLS), f32)
    for p in PACKS:
        qi = QSETS.index(p.qset)
        for j, k in enumerate(p.band_ids):
            b = BANDS[k]
            bw, d = b.bw, 2 * b.bw
            W = np.asarray(inputs[f"g{b.g}_W"][b.i], f32)        # (d, C)
            gam = np.asarray(inputs[f"g{b.g}_gamma"][b.i], f32)  # (d,)
            bet = np.asarray(inputs[f"g{b.g}_beta"][b.i], f32)
            bias = np.asarray(inputs[f"g{b.g}_b"][b.i], f32)     # (C,)
            Wg = gam[:, None] * W
            Wg = Wg - Wg.mean(0, keepdims=True)  # fold -mean(q)*u into Wg
            cols = slice(k * C, (k + 1) * C)
            # device row layout is plane-major: row pl*F2 + off_j + f
            # holds (plane pl, bin f) of band j == torch feature 2f+pl
            for pl in range(2):
                r0 = pl * p.F2 + p.offs[j]
                wextall[r0:r0 + bw, cols] = Wg[2 * np.arange(bw) + pl]
                selall[r0:r0 + bw, 32 * p.pid + j] = 1.0 / d
                erepall[32 * p.quad + j,
                        qi * EREP_COLS + r0:qi * EREP_COLS + r0 + bw] = 1.0
            wextall[p.F, cols] = bias + bet @ W
    return {"wextall": wextall.astype(np.float16),
            "selall": selall.astype(np.float16),
            "erepall": erepall.astype(np.float16)}


# ------------------------------------------------------------ device build --
_CACHE = {}


def _build():
    if "nc" in _CACHE:
        return _CACHE["nc"]
    import concourse.bacc as bacc
    import concourse.tile as tile
    from concourse import mybir

    Alu = mybir.AluOpType
    Act = mybir.ActivationFunctionType
    F32 = mybir.dt.float32
    F32R = mybir.dt.float32r
    F16 = mybir.dt.float16

    def r(ap):  # float32r view for PE streaming of fp32 data
        return ap.bitcast(F32R)

    DBG = int(_os.environ.get("K_DEBUG_SRSTD", "0"))
    nc = bacc.Bacc("TRN2", target_bir_lowering=False, debug=False, num_devices=8)
    x_d = nc.dram_tensor("x", [2, F_BINS, T], F16, kind="ExternalInput")
    out_d = nc.dram_tensor("out", [T, 31, C], F16, kind="ExternalOutput")
    dbg_d = None
    if DBG:
        dbg_d = nc.dram_tensor("dbg", [6, 96, SPAN], F32, kind="ExternalOutput")
    DBGQ = int(_os.environ.get("K_DEBUG_Q", "0"))
    dbgq_d = None
    if DBGQ:
        dbgq_d = nc.dram_tensor("dbgq", [6, 128, SPAN], F16,
                                kind="ExternalOutput")
    wext_d = nc.dram_tensor("wextall", [128, OB_COLS], F16, kind="ExternalInput")
    selx_d = nc.dram_tensor("selall", [128, 32 * len(PACKS)], F16,
                            kind="ExternalInput")
    erep_d = nc.dram_tensor("erepall", [96, 2 * EREP_COLS], F16,
                            kind="ExternalInput")

    with tile.TileContext(nc) as tc:
        with (
            tc.tile_pool(name="const", bufs=1) as const,
            tc.tile_pool(name="xsqp", bufs=7) as xsqp,
            tc.tile_pool(name="stt", bufs=2) as stt,
            tc.tile_pool(name="obp",
                         bufs=int(_os.environ.get("K_OBP", "3"))) as obp,
            tc.tile_pool(name="srp", bufs=2) as srp,
            tc.tile_pool(name="stps", bufs=1, space="PSUM") as stps,
            tc.tile_pool(name="outps", bufs=4, space="PSUM") as outps,
        ):
            # ---- resident constants: tiny stats params, first x slab, wext.
            selall = const.tile([128, 32 * len(PACKS)], F16, tag="sel", name="selall")
            nc.sync.dma_start(out=selall[:], in_=selx_d[:])
            erepall = const.tile([96, 2 * EREP_COLS], F16, tag="er", name="erepall")
            nc.sync.dma_start(out=erepall[:], in_=erep_d[:])
            xin = {}
            for p in PACKS:
                xin[p.pid] = const.tile([p.F, T], F16, tag=f"xin{p.pid}",
                                        name=f"xin{p.pid}")

            SLAB3D = int(_os.environ.get("K_SLAB3D", "0"))

            def load_slab(s0, sw):
                for p in PACKS:
                    if SLAB3D:
                        # one DMA per pack covering both planes: src is a 3D
                        # [2, F2, sw] DRAM AP, dst a plane-major [F, sw] AP
                        s_ = x_d[:, p.f0:p.f0 + p.F2, s0:s0 + sw]
                        d_ = xin[p.pid][0:p.F, s0:s0 + sw]
                        nc.sync.dma_start(out=d_, in_=s_)
                    else:
                        for pl in range(2):
                            s_ = x_d[pl, p.f0:p.f0 + p.F2, s0:s0 + sw]
                            d_ = xin[p.pid][pl * p.F2:(pl + 1) * p.F2,
                                            s0:s0 + sw]
                            nc.sync.dma_start(out=d_, in_=s_)

            # startup: first group covers span 0 only so stats can start
            # ASAP; wext next (span-0 mains need it); then the span-1 slab.
            _groups = [int(g) for g in SLAB0W.split(",") if g]
            assert sum(_groups) <= T
            wext = const.tile([128, OB_COLS], F16, tag="wx", name="wextall")
            _g0 = 0
            for _gi, _gw in enumerate(_groups):
                load_slab(_g0, _gw)
                _g0 += _gw
                if _gi == 0:
                    nc.sync.dma_start(out=wext[:], in_=wext_d[:])
            _s0w = 0  # spans fully covered by startup groups
            _acc = 0
            for _s0, _sw in SPANS:
                if _acc + _sw <= _g0:
                    _acc += _sw
                    _s0w += 1
                else:
                    break

            def selx(p):
                return selall[0:p.F, 32 * p.pid:32 * p.pid + 32]

            def erep(p):
                qi = QSETS.index(p.qset)
                q0 = 32 * p.quad
                return erepall[q0:q0 + p.n,
                               qi * EREP_COLS:qi * EREP_COLS + p.F]

            eps_t = const.tile([128, 1], F32, tag="epsc", name="epsc")
            nc.vector.memset(eps_t[:], EPS)
            ones_t = const.tile([32, SPAN], F16, tag="ones", name="ones")
            nc.vector.memset(ones_t[:], 1.0)
            # PE p-state warm-up: keep the PE busy during the initial x-slab
            # load so span-0 matmuls run at full clock.
            for _ in range(WARMUP):
                wm = outps.tile([128, 512], F32, tag="op", name="wm")
                nc.tensor.matmul(
                    wm[0:128, 0:192], ones_t[0:32, 0:128],
                    ones_t[0:32, 0:192], start=True, stop=True)
            if DUMMY_SQRT:
                # dummy Sqrt: makes the compiler pick the sqrt-bearing
                # activation table up front (it also holds Square/Copy), so
                # no table reload lands on the span-0 critical chain.
                scr0 = const.tile([1, 1], F32, tag="scr0", name="scr0")
                nc.scalar.activation(scr0[0:1, 0:1], eps_t[0:1, 0:1],
                                     Act.Sqrt, bias=eps_t[0:1, 0:1],
                                     scale=1.0)

            # persistent double-buffered lhsT tiles; the ones row (row F)
            # is written once, q rows get rewritten every span.
            # engine partition writes must start 32-aligned, so the ones
            # row (row F) is filled by a 32-row aligned copy; the x rows it
            # clobbers are rewritten by the per-span q prep before use.
            xpt = {}
            for i, p in enumerate(PACKS):
                for par in range(2):
                    t_ = const.tile([p.K32, SPAN], F16, tag=f"xp{p.pid}_{par}",
                                    name=f"xp{p.pid}_{par}")
                    xpt[(p.pid, par)] = t_
                    for m0 in range(p.F // 32 * 32, p.K32, 32):
                        if (i + par) % 2 == 0:
                            nc.vector.tensor_copy(t_[m0:m0 + 32, :],
                                                  ones_t[0:32, :])
                        else:
                            nc.scalar.activation(t_[m0:m0 + 32, :],
                                                 ones_t[0:32, :], Act.Copy)

            from concourse.dve_ops import (
                RECIP_APPROX_FAST_CONSTS as _RC,
                RECIPROCAL_APPROX_FAST as _RF,
            )
            srstd_by_si = {}

            def emit_stats(si):
                """Stats + rstd chain for span si.  Called one span AHEAD
                (software pipelining) so srstd is ready when the span's
                prep/mains start.  fp16 matmuls may write PSUM at 32-aligned
                bases, so per-pack sums land directly in the quadrant rows
                of per-qset [96, SPAN] PSUM tiles."""
                s0, sw = SPANS[si]
                srstd = {qs: srp.tile([96, SPAN], F32, tag=f"sr{qs}",
                                      name=f"sr{qs}") for qs in QSETS}
                srstd16 = {qs: srp.tile([96, SPAN], F16, tag=f"sr16{qs}",
                                        name=f"sr16{qs}") for qs in QSETS}
                srstd_by_si[si] = srstd16
                mu_ps = {qs: stps.tile([96, SPAN], F32, tag=f"mu{qs}",
                                       name=f"mu{qs}") for qs in QSETS}
                msq_ps = {qs: stps.tile([96, SPAN], F32, tag=f"ms{qs}",
                                        name=f"ms{qs}") for qs in QSETS}
                for p in PACKS_IL:
                    q0 = 32 * p.quad
                    xin_f = xin[p.pid][:, s0:s0 + sw]
                    # x^2 on GPSIMD (SBUF->SBUF is legal there), freeing the
                    # scalar engine for PSUM drains
                    xsq = xsqp.tile([128, SPAN], F16, tag="xsq", name="xsq")
                    nc.gpsimd.tensor_tensor(
                        xsq[0:p.F, :sw], xin_f, xin_f, op=Alu.mult)
                    nc.tensor.matmul(
                        mu_ps[p.qset][q0:q0 + 32, :sw], selx(p),
                        xin_f, start=True, stop=True)
                    nc.tensor.matmul(
                        msq_ps[p.qset][q0:q0 + 32, :sw], selx(p),
                        xsq[0:p.F, :sw], start=True, stop=True)
                # ---- B) rstd = rsqrt(msq - mu^2 + eps), batched per set
                for qs in QSETS:
                    musq = stt.tile([96, SPAN], F32, tag=f"musq{qs}",
                                    name=f"musq{qs}")
                    nc.scalar.activation(
                        musq[0:96, :sw], mu_ps[qs][0:96, :sw], Act.Square)
                    var = stt.tile([96, SPAN], F32, tag="var", name="var")
                    nc.vector.tensor_tensor(
                        var[0:96, :sw], msq_ps[qs][0:96, :sw],
                        musq[0:96, :sw], op=Alu.subtract)
                    sq = stt.tile([96, SPAN], F32, tag="sq", name="sq")
                    nc.scalar.activation(
                        sq[0:96, :sw], var[0:96, :sw], Act.Sqrt,
                        bias=eps_t[0:96, 0:1], scale=1.0)
                    # reciprocal_approx_fast with an fp32r-rounded output
                    # view (its wrapper asserts fp32, so call the custom-DVE
                    # op directly; the seed trick only needs fp32 INPUT bits)
                    nc.vector._custom_dve(
                        _RF, out=srstd[qs][0:96, :sw],
                        in0=sq[0:96, :sw], s0=_RC["s0"], s1=_RC["s1"],
                        imm2=_RC["imm2"])
                    # fp16 copy for the (all-fp16) erep broadcast matmul;
                    # on GPSIMD to keep Act/DVE free for drains
                    nc.gpsimd.tensor_copy(srstd16[qs][0:96, :sw],
                                          srstd[qs][0:96, :sw])
                    if DBG and si == 0:
                        qi = QSETS.index(qs)
                        # dump mu^2, var, srstd for span 0
                        nc.sync.dma_start(out=dbg_d[3 * qi + 0, :, :sw],
                                          in_=musq[0:96, :sw])
                        nc.sync.dma_start(out=dbg_d[3 * qi + 1, :, :sw],
                                          in_=var[0:96, :sw])
                        nc.sync.dma_start(out=dbg_d[3 * qi + 2, :, :sw],
                                          in_=srstd[qs][0:96, :sw])

            def emit_prep(si):
                """Pack lhsT prep for span si: q = x * rstd_rep, into the
                parity-si xpt tiles.  Emitted during span si-1 (the tiles are
                double-buffered) so span si's first mains start immediately."""
                s0, sw = SPANS[si]
                par = si % 2
                srstd = srstd_by_si.pop(si)
                for p in PACKS_IL:
                    q0 = 32 * p.quad
                    t_ = xpt[(p.pid, par)]
                    rr = outps.tile([128, 512], F32, tag="op", name="rr")
                    nc.tensor.matmul(
                        rr[0:p.F, :sw],
                        erep(p),
                        srstd[p.qset][q0:q0 + p.n, :sw],
                        start=True, stop=True)
                    nc.vector.tensor_tensor(
                        t_[0:p.F, :sw], xin[p.pid][:, s0:s0 + sw],
                        rr[0:p.F, :sw], op=Alu.mult)
                    if DBGQ and si == 0:
                        nc.sync.dma_start(out=dbgq_d[p.pid, 0:p.K32, :sw],
                                          in_=t_[0:p.K32, :sw])

            emit_stats(0)
            emit_prep(0)
            for si, (s0, sw) in enumerate(SPANS):
                par = si % 2
                # ---- D) main matmuls + drain + one DMA per t-chunk; the
                # next span's slab/stats/prep interleave between chunks so
                # the in-order engine queues never batch them behind a whole
                # span of drains.
                for ci, c0 in enumerate(range(s0, s0 + sw, CHUNK)):
                    cw = min(CHUNK, s0 + sw - c0)
                    ob = obp.tile([128, OB_COLS], F16, tag="ob", name="ob")
                    di = 0
                    for p in PACKS:
                        lhsT = xpt[(p.pid, par)][0:p.K, c0 - s0:c0 - s0 + cw]
                        obase = p.k0 * C
                        for (n0, nw) in _blocks(p):
                            op = outps.tile([128, 512], F32, tag="op", name="op")
                            nc.tensor.matmul(
                                op[0:cw, 0:nw], lhsT,
                                wext[0:p.K, obase + n0:obase + n0 + nw],
                                start=True, stop=True)
                            eng = DRAIN_ENG[di]
                            dst = ob[0:cw, obase + n0:obase + n0 + nw]
                            if eng == "scalar":
                                nc.scalar.activation(dst, op[0:cw, 0:nw],
                                                     Act.Copy)
                            else:
                                nc.vector.tensor_copy(dst, op[0:cw, 0:nw])
                            di += 1
                    nc.sync.dma_start(
                        out=out_d[c0:c0 + cw, :, :].rearrange(
                            "t j c -> t (j c)"),
                        in_=ob[0:cw, :])
                    if ci == 0 and _s0w <= si + 2 < len(SPANS):
                        load_slab(*SPANS[si + 2])
                    if ci == 1 and si + 1 < len(SPANS):
                        emit_stats(si + 1)
                if si + 1 < len(SPANS):
                    emit_prep(si + 1)

    nc.compile()
    _CACHE["nc"] = nc
    return nc


# ------------------------------------------------------------------ driver --
def kernel(**inputs):
    from concourse.bass_utils import run_bass_kernel_spmd

    x = np.asarray(inputs["x"]).astype(np.float16)
    B = x.shape[0]
    assert x.shape == (8, 2, F_BINS, T)
    ext = _host_params(inputs)
    nc = _build()
    in_maps = []
    for b in range(B):
        m = {"x": x[b]}
        m.update(ext)
        in_maps.append(m)
    res = run_bass_kernel_spmd(nc, in_maps, core_ids=list(range(8)))
    out = np.stack([res.results[b]["out"].transpose(1, 0, 2)
                    for b in range(B)], axis=0)
    return out.astype(np.float32)
